# revision 7
# baseline (speedup 1.0000x reference)
"""MoE layer (B=8,T=1024,D=512,F=2048,E=8,top-2) on 8 NeuronCores.

Strategy (expert parallel, per the sharding hint):
- Host computes the router (logits -> softmax -> top-2 -> combine weights);
  that routing defines the sharding: tokens are gathered per expert and
  dispatched to the core owning that expert (the "all-to-all by routing
  assignment" happens in the host gather/scatter).
- Core e runs the expert-e FFN over its gathered tokens:
      y = relu(x @ W1[e] + b1[e]) @ W2[e], scaled per-token by the combine
  weight. Matmuls run in f32r (full PE rate, ~11-bit mantissa), accumulation
  in fp32 PSUM.
- Host scatter-adds the per-expert outputs back (plus the cw-weighted b2
  rank-1 term) into the full (B,T,D) output.
"""

import os
import numpy as np

import concourse.bass as bass
from bass_rust import add_dep_helper
import concourse.tile as tile
from concourse import bacc, mybir
from concourse.bass_utils import run_bass_kernel_spmd

F32 = mybir.dt.float32
F32R = mybir.dt.float32r
F16 = mybir.dt.float16

B, T, D, F, E, TOPK = 8, 1024, 512, 2048, 8, 2
N = B * T
P = 128
N_CORES = 8
KT1 = D // P    # 4  k-tiles for x @ W1
KT2 = F // P    # 16 k-tiles for h @ W2
FT = F // P     # 16 f-tiles of hT


def _chunks(C):
    """Split token capacity C into free-dim chunks (<=512, multiples of 128)."""
    out = []
    c0 = 0
    while c0 < C:
        s = min(512, C - c0)
        out.append((c0, s))
        c0 += s
    return out


def _build(C):
    nc = bacc.Bacc()
    Ct = C // P

    xt_d = nc.dram_tensor("xt", [D, C], F16, kind="ExternalInput")
    w1_d = nc.dram_tensor("w1", [D, F], F16, kind="ExternalInput")
    w2_d = nc.dram_tensor("w2", [F, D], F16, kind="ExternalInput")
    b1_d = nc.dram_tensor("b1", [P, FT], F32, kind="ExternalInput")
    cw_d = nc.dram_tensor("cw", [P, Ct], F32, kind="ExternalInput")
    y_d = nc.dram_tensor("y", [C, D], F32, kind="ExternalOutput")

    chunks = _chunks(C)

    with tile.TileContext(nc) as tc:
        with (
            tc.tile_pool(name="weights", bufs=1) as wpool,
            tc.tile_pool(name="xt", bufs=1) as xpool,
            tc.tile_pool(name="h", bufs=2 * FT + 1) as hpool,
            tc.tile_pool(name="y", bufs=4) as ypool,
            tc.tile_pool(name="psh", bufs=3, space="PSUM") as psh,
            tc.tile_pool(name="psy", bufs=3, space="PSUM") as psy,
        ):
            # ---- tiles ----
            w1_t = wpool.tile([P, KT1 * F], F16, tag="w1")
            w1_v = w1_t[:].rearrange("p (kt f) -> p kt f", kt=KT1)
            w1_src = w1_d.rearrange("(kt p) f -> p kt f", p=P)
            w2_t = wpool.tile([P, KT2 * D], F16, tag="w2")
            b1_t = wpool.tile([P, FT], F32, tag="b1")
            cw_t = wpool.tile([P, Ct], F32, tag="cw")
            xt_t = xpool.tile([P, KT1 * C], F16, tag="xt")
            xt_v = xt_t[:].rearrange("p (kt c) -> p kt c", kt=KT1)
            xt_src = xt_d.rearrange("(kt p) c -> p kt c", p=P)

            # Sync queue: what mm1 needs first (w1 quarters, then xt chunks,
            # interleaved so chunk-0 compute starts as early as possible).
            FQ = FT // 4
            def w1_dma(q):
                nc.sync.dma_start(
                    w1_v[:, :, q * FQ * P : (q + 1) * FQ * P],
                    w1_src[:, :, q * FQ * P : (q + 1) * FQ * P],
                )
            w1_dma(0)
            xt0_dma = nc.sync.dma_start(
                xt_v[:, :, 0 : chunks[0][1]], xt_src[:, :, 0 : chunks[0][1]]
            )
            for q in range(1, 4):
                w1_dma(q)
            for c0, S in chunks[1:]:
                nc.sync.dma_start(
                    xt_v[:, :, c0 : c0 + S], xt_src[:, :, c0 : c0 + S]
                )

            # GpSimd queue: everything whose deadline is later (b1 for the
            # first relu, w2 for mm2, cw for the y scale). w2 is big; gate it
            # on xt0's completion so it doesn't halve the HBM bandwidth
            # during the startup window the PE is waiting on.
            nc.gpsimd.dma_start(b1_t[:], b1_d[:])
            w2_dma = nc.gpsimd.dma_start(
                w2_t[:].rearrange("p (kt d) -> p kt d", kt=KT2),
                w2_d.rearrange("(kt p) d -> p kt d", p=P),
            )
            add_dep_helper(w2_dma.ins, xt0_dma.ins, sync=True,
                           reason="defer w2 until xt0 landed")
            nc.gpsimd.dma_start(cw_t[:], cw_d[:])

            # ---- PE warm-up: junk matmuls on a zeroed tile while the input
            # DMAs stream, so the HAM clock-gate reaches 8/8 before real work
            # and the first real matmul doesn't pay the cold 1.2 GHz window.
            warm = wpool.tile([P, 512], F16, tag="warm")
            nc.gpsimd.memset(warm[:], 0.0)
            wps = psy.tile([P, 512], F32, tag="psy")
            for _ in range(24):
                chain_warm = nc.tensor.matmul(
                    warm[:, 0:P], warm[:], start=True, stop=True, out=None
                ) if False else nc.tensor.matmul(
                    wps[:], warm[:, 0:P], warm[:], start=True, stop=True
                )
            del chain_warm

            # ---- software-pipelined chunk loop: mm1(ci) then mm2(ci-1) ----
            h_tiles = {}  # chunk idx -> list of FT hT tiles
            prev_grp = [None, None]  # previous group's first MM, current group's first MM

            def group_start():
                prev_grp[0], prev_grp[1] = prev_grp[1], None

            def chain(bi):
                # Pin PE group issue order to program order (first-MM to
                # first-MM): the scheduler otherwise reorders independent
                # matmul groups ahead of ready ones and stalls the PE on
                # not-yet-DMA'd data. Within-group order is already enforced
                # by PSUM accumulation, so leave those edges free for
                # LDWEIGHTS pull-ahead.
                if prev_grp[1] is None:
                    prev_grp[1] = bi
                    if prev_grp[0] is not None:
                        add_dep_helper(bi.ins, prev_grp[0].ins, sync=False,
                                       reason="PE group-order chain")

            def mm1(ci):
                c0, S = chunks[ci]
                tiles = []
                for fi in range(FT):
                    group_start()
                    ph = psh.tile([P, S], F32, tag="psh")
                    for kt in range(KT1):
                        chain(nc.tensor.matmul(
                            ph[:],
                            w1_t[:, kt * F + fi * P : kt * F + (fi + 1) * P],
                            xt_v[:, kt, c0 : c0 + S],
                            start=(kt == 0),
                            stop=(kt == KT1 - 1),
                        ))
                    ht = hpool.tile([P, S], F16, tag="h")
                    nc.scalar.activation(
                        ht[:],
                        ph[:],
                        mybir.ActivationFunctionType.Relu,
                        bias=b1_t[:, fi : fi + 1],
                    )
                    tiles.append(ht)
                h_tiles[ci] = tiles

            def mm2(ci):
                c0, S = chunks[ci]
                tiles = h_tiles.pop(ci)
                for mi in range(S // P):
                    group_start()
                    py = psy.tile([P, D], F32, tag="psy")
                    for kt in range(KT2):
                        chain(nc.tensor.matmul(
                            py[:],
                            tiles[kt][:, mi * P : (mi + 1) * P],
                            w2_t[:, kt * D : (kt + 1) * D],
                            start=(kt == 0),
                            stop=(kt == KT2 - 1),
                        ))
                    yt = ypool.tile([P, D], F32, tag="y")
                    ct = c0 // P + mi
                    nc.vector.tensor_scalar_mul(yt[:], py[:], cw_t[:, ct : ct + 1])
                    nc.gpsimd.dma_start(y_d[ct * P : (ct + 1) * P, :], yt[:])

            for ci in range(len(chunks) + 1):
                if ci < len(chunks):
                    mm1(ci)
                if ci >= 1:
                    mm2(ci - 1)

    nc.compile()
    return nc


def kernel(x, Wr, br, W1, b1, W2, b2):
    x = np.ascontiguousarray(np.asarray(x, np.float32))
    Wr = np.asarray(Wr, np.float32)
    br = np.asarray(br, np.float32)
    W1 = np.ascontiguousarray(np.asarray(W1, np.float32))
    b1 = np.ascontiguousarray(np.asarray(b1, np.float32))
    W2 = np.ascontiguousarray(np.asarray(W2, np.float32))
    b2 = np.asarray(b2, np.float32)

    xf = x.reshape(N, D)

    # ---- host router: softmax -> top-2 -> combine weights ----
    logits = xf @ Wr + br
    m = logits.max(axis=-1, keepdims=True)
    p = np.exp(logits - m, dtype=np.float32)
    p /= p.sum(axis=-1, keepdims=True)
    idx = np.argpartition(-p, TOPK - 1, axis=-1)[:, :TOPK]  # top-2 experts
    cw = np.zeros((N, E), np.float32)
    np.put_along_axis(cw, idx, np.take_along_axis(p, idx, axis=-1), axis=-1)

    tok = [np.nonzero(cw[:, e] > 0)[0] for e in range(E)]
    counts = [len(t) for t in tok]
    C = max(256, -(-max(counts) // 256) * 256)  # capacity, multiple of 256

    in_maps = []
    for e in range(E):
        te, ce = tok[e], counts[e]
        xt = np.zeros((D, C), np.float16)
        xt[:, :ce] = xf[te].T
        cwe = np.zeros((C,), np.float32)
        cwe[:ce] = cw[te, e]
        in_maps.append(
            {
                "xt": xt,
                "w1": np.ascontiguousarray(W1[e], np.float16),
                "w2": np.ascontiguousarray(W2[e], np.float16),
                "b1": np.ascontiguousarray(b1[e].reshape(FT, P).T),
                "cw": np.ascontiguousarray(cwe.reshape(C // P, P).T),
            }
        )

    nc = _build(C)
    trace = bool(os.environ.get("BASS_MOE_TRACE"))
    res = run_bass_kernel_spmd(
        nc,
        in_maps,
        core_ids=list(range(N_CORES)),
        trace=trace,
        trace_cores=list(range(N_CORES)) if trace else None,
    )
    if trace and res.exec_time_ns is not None:
        print(f"HW exec time: {res.exec_time_ns} ns")
        print(f"mean exec time: {res.mean_exec_time_ns} ns")
        if res.instructions_and_trace is not None:
            print(f"trace: {res.instructions_and_trace[1]}")

    # ---- host combine: scatter-add expert outputs + cw-weighted b2 ----
    out = cw @ b2  # (N, D) rank-E update: sum_e cw[:,e] * b2[e]
    for e in range(E):
        out[tok[e]] += res.results[e]["y"][: counts[e]]
    return out.reshape(B, T, D)


# revision 8
# speedup vs baseline: 1.0060x; 1.0060x over previous
"""MoE layer (B=8,T=1024,D=512,F=2048,E=8,top-2) on 8 NeuronCores.

Strategy (expert parallel, per the sharding hint):
- Host computes the router (logits -> softmax -> top-2 -> combine weights);
  that routing defines the sharding: tokens are gathered per expert and
  dispatched to the core owning that expert (the "all-to-all by routing
  assignment" happens in the host gather/scatter).
- Core e runs the expert-e FFN over its gathered tokens:
      y = relu(x @ W1[e] + b1[e]) @ W2[e], scaled per-token by the combine
  weight. Matmuls run in f32r (full PE rate, ~11-bit mantissa), accumulation
  in fp32 PSUM.
- Host scatter-adds the per-expert outputs back (plus the cw-weighted b2
  rank-1 term) into the full (B,T,D) output.
"""

import os
import numpy as np

import concourse.bass as bass
from bass_rust import add_dep_helper
import concourse.tile as tile
from concourse import bacc, mybir
from concourse.bass_utils import run_bass_kernel_spmd

F32 = mybir.dt.float32
F32R = mybir.dt.float32r
F16 = mybir.dt.float16

B, T, D, F, E, TOPK = 8, 1024, 512, 2048, 8, 2
N = B * T
P = 128
N_CORES = 8
KT1 = D // P    # 4  k-tiles for x @ W1
KT2 = F // P    # 16 k-tiles for h @ W2
FT = F // P     # 16 f-tiles of hT


def _chunks(C):
    """Split token capacity C into free-dim chunks (<=512, multiples of 128)."""
    out = []
    c0 = 0
    while c0 < C:
        s = min(512, C - c0)
        out.append((c0, s))
        c0 += s
    return out


def _build(C):
    nc = bacc.Bacc()
    Ct = C // P

    xt_d = nc.dram_tensor("xt", [D, C], F16, kind="ExternalInput")
    w1_d = nc.dram_tensor("w1", [D, F], F16, kind="ExternalInput")
    w2_d = nc.dram_tensor("w2", [F, D], F16, kind="ExternalInput")
    b1_d = nc.dram_tensor("b1", [P, FT], F32, kind="ExternalInput")
    cw_d = nc.dram_tensor("cw", [P, Ct], F32, kind="ExternalInput")
    y_d = nc.dram_tensor("y", [C, D], F32, kind="ExternalOutput")

    chunks = _chunks(C)

    with tile.TileContext(nc) as tc:
        with (
            tc.tile_pool(name="weights", bufs=1) as wpool,
            tc.tile_pool(name="xt", bufs=1) as xpool,
            tc.tile_pool(name="h", bufs=2 * FT + 1) as hpool,
            tc.tile_pool(name="y", bufs=4) as ypool,
            tc.tile_pool(name="psh", bufs=3, space="PSUM") as psh,
            tc.tile_pool(name="psy", bufs=3, space="PSUM") as psy,
        ):
            # ---- tiles ----
            w1_t = wpool.tile([P, KT1 * F], F16, tag="w1")
            w1_v = w1_t[:].rearrange("p (kt f) -> p kt f", kt=KT1)
            w1_src = w1_d.rearrange("(kt p) f -> p kt f", p=P)
            w2_t = wpool.tile([P, KT2 * D], F16, tag="w2")
            b1_t = wpool.tile([P, FT], F32, tag="b1")
            cw_t = wpool.tile([P, Ct], F32, tag="cw")
            xt_t = xpool.tile([P, KT1 * C], F16, tag="xt")
            xt_v = xt_t[:].rearrange("p (kt c) -> p kt c", kt=KT1)
            xt_src = xt_d.rearrange("(kt p) c -> p kt c", p=P)

            # PE warm-up: a few junk matmuls on a zeroed tile while the input
            # DMAs stream, so the HAM clock-gate reaches 8/8 before real work
            # arrives and the first real matmuls don't run in the cold
            # 1.2 GHz window. Emitted before the DMA issues so the memset is
            # first in the GpSimd stream.
            warm = wpool.tile([P, 512], F16, tag="warm")
            nc.gpsimd.memset(warm[:], 0.0)
            wps = psy.tile([P, 512], F32, tag="psy")
            for _ in range(6):
                nc.tensor.matmul(wps[:], warm[:, 0:P], warm[:], start=True, stop=True)

            # Sync queue: what mm1 needs first (w1 quarters, then xt chunks,
            # interleaved so chunk-0 compute starts as early as possible).
            FQ = FT // 4
            def w1_dma(q):
                nc.sync.dma_start(
                    w1_v[:, :, q * FQ * P : (q + 1) * FQ * P],
                    w1_src[:, :, q * FQ * P : (q + 1) * FQ * P],
                )
            w1_dma(0)
            xt0_dma = nc.sync.dma_start(
                xt_v[:, :, 0 : chunks[0][1]], xt_src[:, :, 0 : chunks[0][1]]
            )
            for q in range(1, 4):
                w1_dma(q)
            for c0, S in chunks[1:]:
                nc.sync.dma_start(
                    xt_v[:, :, c0 : c0 + S], xt_src[:, :, c0 : c0 + S]
                )

            # GpSimd queue: everything whose deadline is later (b1 for the
            # first relu, w2 for mm2, cw for the y scale). w2 is big; gate it
            # on xt0's completion so it doesn't halve the HBM bandwidth
            # during the startup window the PE is waiting on.
            nc.gpsimd.dma_start(b1_t[:], b1_d[:])
            w2_dma = nc.gpsimd.dma_start(
                w2_t[:].rearrange("p (kt d) -> p kt d", kt=KT2),
                w2_d.rearrange("(kt p) d -> p kt d", p=P),
            )
            add_dep_helper(w2_dma.ins, xt0_dma.ins, sync=True,
                           reason="defer w2 until xt0 landed")
            nc.gpsimd.dma_start(cw_t[:], cw_d[:])

            # ---- software-pipelined chunk loop: mm1(ci) then mm2(ci-1) ----
            h_tiles = {}  # chunk idx -> list of FT hT tiles
            prev_grp = [None, None]  # previous group's first MM, current group's first MM

            def group_start():
                prev_grp[0], prev_grp[1] = prev_grp[1], None

            def chain(bi):
                # Pin PE group issue order to program order (first-MM to
                # first-MM): the scheduler otherwise reorders independent
                # matmul groups ahead of ready ones and stalls the PE on
                # not-yet-DMA'd data. Within-group order is already enforced
                # by PSUM accumulation, so leave those edges free for
                # LDWEIGHTS pull-ahead.
                if prev_grp[1] is None:
                    prev_grp[1] = bi
                    if prev_grp[0] is not None:
                        add_dep_helper(bi.ins, prev_grp[0].ins, sync=False,
                                       reason="PE group-order chain")

            def mm1(ci):
                c0, S = chunks[ci]
                tiles = []
                for fi in range(FT):
                    group_start()
                    ph = psh.tile([P, S], F32, tag="psh")
                    for kt in range(KT1):
                        chain(nc.tensor.matmul(
                            ph[:],
                            w1_t[:, kt * F + fi * P : kt * F + (fi + 1) * P],
                            xt_v[:, kt, c0 : c0 + S],
                            start=(kt == 0),
                            stop=(kt == KT1 - 1),
                        ))
                    ht = hpool.tile([P, S], F16, tag="h")
                    nc.scalar.activation(
                        ht[:],
                        ph[:],
                        mybir.ActivationFunctionType.Relu,
                        bias=b1_t[:, fi : fi + 1],
                    )
                    tiles.append(ht)
                h_tiles[ci] = tiles

            def mm2(ci):
                c0, S = chunks[ci]
                tiles = h_tiles.pop(ci)
                for mi in range(S // P):
                    group_start()
                    py = psy.tile([P, D], F32, tag="psy")
                    for kt in range(KT2):
                        chain(nc.tensor.matmul(
                            py[:],
                            tiles[kt][:, mi * P : (mi + 1) * P],
                            w2_t[:, kt * D : (kt + 1) * D],
                            start=(kt == 0),
                            stop=(kt == KT2 - 1),
                        ))
                    yt = ypool.tile([P, D], F32, tag="y")
                    ct = c0 // P + mi
                    nc.vector.tensor_scalar_mul(yt[:], py[:], cw_t[:, ct : ct + 1])
                    nc.gpsimd.dma_start(y_d[ct * P : (ct + 1) * P, :], yt[:])

            for ci in range(len(chunks) + 1):
                if ci < len(chunks):
                    mm1(ci)
                if ci >= 1:
                    mm2(ci - 1)

    nc.compile()
    return nc


def kernel(x, Wr, br, W1, b1, W2, b2):
    x = np.ascontiguousarray(np.asarray(x, np.float32))
    Wr = np.asarray(Wr, np.float32)
    br = np.asarray(br, np.float32)
    W1 = np.ascontiguousarray(np.asarray(W1, np.float32))
    b1 = np.ascontiguousarray(np.asarray(b1, np.float32))
    W2 = np.ascontiguousarray(np.asarray(W2, np.float32))
    b2 = np.asarray(b2, np.float32)

    xf = x.reshape(N, D)

    # ---- host router: softmax -> top-2 -> combine weights ----
    logits = xf @ Wr + br
    m = logits.max(axis=-1, keepdims=True)
    p = np.exp(logits - m, dtype=np.float32)
    p /= p.sum(axis=-1, keepdims=True)
    idx = np.argpartition(-p, TOPK - 1, axis=-1)[:, :TOPK]  # top-2 experts
    cw = np.zeros((N, E), np.float32)
    np.put_along_axis(cw, idx, np.take_along_axis(p, idx, axis=-1), axis=-1)

    tok = [np.nonzero(cw[:, e] > 0)[0] for e in range(E)]
    counts = [len(t) for t in tok]
    C = max(256, -(-max(counts) // 256) * 256)  # capacity, multiple of 256

    in_maps = []
    for e in range(E):
        te, ce = tok[e], counts[e]
        xt = np.zeros((D, C), np.float16)
        xt[:, :ce] = xf[te].T
        cwe = np.zeros((C,), np.float32)
        cwe[:ce] = cw[te, e]
        in_maps.append(
            {
                "xt": xt,
                "w1": np.ascontiguousarray(W1[e], np.float16),
                "w2": np.ascontiguousarray(W2[e], np.float16),
                "b1": np.ascontiguousarray(b1[e].reshape(FT, P).T),
                "cw": np.ascontiguousarray(cwe.reshape(C // P, P).T),
            }
        )

    nc = _build(C)
    trace = bool(os.environ.get("BASS_MOE_TRACE"))
    res = run_bass_kernel_spmd(
        nc,
        in_maps,
        core_ids=list(range(N_CORES)),
        trace=trace,
        trace_cores=list(range(N_CORES)) if trace else None,
    )
    if trace and res.exec_time_ns is not None:
        print(f"HW exec time: {res.exec_time_ns} ns")
        print(f"mean exec time: {res.mean_exec_time_ns} ns")
        if res.instructions_and_trace is not None:
            print(f"trace: {res.instructions_and_trace[1]}")

    # ---- host combine: scatter-add expert outputs + cw-weighted b2 ----
    out = cw @ b2  # (N, D) rank-E update: sum_e cw[:,e] * b2[e]
    for e in range(E):
        out[tok[e]] += res.results[e]["y"][: counts[e]]
    return out.reshape(B, T, D)


# revision 9
# speedup vs baseline: 1.0146x; 1.0085x over previous
"""MoE layer (B=8,T=1024,D=512,F=2048,E=8,top-2) on 8 NeuronCores.

Strategy (expert parallel, per the sharding hint):
- Host computes the router (logits -> softmax -> top-2 -> combine weights);
  that routing defines the sharding: tokens are gathered per expert and
  dispatched to the core owning that expert (the "all-to-all by routing
  assignment" happens in the host gather/scatter).
- Core e runs the expert-e FFN over its gathered tokens:
      y = relu(x @ W1[e] + b1[e]) @ W2[e], scaled per-token by the combine
  weight. Matmuls run in f32r (full PE rate, ~11-bit mantissa), accumulation
  in fp32 PSUM.
- Host scatter-adds the per-expert outputs back (plus the cw-weighted b2
  rank-1 term) into the full (B,T,D) output.
"""

import os
import numpy as np

import concourse.bass as bass
from bass_rust import add_dep_helper
import concourse.tile as tile
from concourse import bacc, mybir
from concourse.bass_utils import run_bass_kernel_spmd

F32 = mybir.dt.float32
F32R = mybir.dt.float32r
F16 = mybir.dt.float16

B, T, D, F, E, TOPK = 8, 1024, 512, 2048, 8, 2
N = B * T
P = 128
N_CORES = 8
KT1 = D // P    # 4  k-tiles for x @ W1
KT2 = F // P    # 16 k-tiles for h @ W2
FT = F // P     # 16 f-tiles of hT


def _chunks(C):
    """Split token capacity C into free-dim chunks (<=512, multiples of 128)."""
    out = []
    c0 = 0
    while c0 < C:
        s = min(512, C - c0)
        out.append((c0, s))
        c0 += s
    return out


def _build(C):
    nc = bacc.Bacc()
    Ct = C // P

    xt_d = nc.dram_tensor("xt", [D, C], F16, kind="ExternalInput")
    w1_d = nc.dram_tensor("w1", [D, F], F16, kind="ExternalInput")
    w2_d = nc.dram_tensor("w2", [F, D], F16, kind="ExternalInput")
    b1_d = nc.dram_tensor("b1", [P, FT], F32, kind="ExternalInput")
    cw_d = nc.dram_tensor("cw", [P, Ct], F32, kind="ExternalInput")
    y_d = nc.dram_tensor("y", [C, D], F32, kind="ExternalOutput")

    chunks = _chunks(C)

    with tile.TileContext(nc) as tc:
        with (
            tc.tile_pool(name="weights", bufs=1) as wpool,
            tc.tile_pool(name="xt", bufs=1) as xpool,
            tc.tile_pool(name="h", bufs=2 * FT + 1) as hpool,
            tc.tile_pool(name="y", bufs=4) as ypool,
            tc.tile_pool(name="psh", bufs=3, space="PSUM") as psh,
            tc.tile_pool(name="psy", bufs=3, space="PSUM") as psy,
        ):
            # ---- tiles ----
            w1_t = wpool.tile([P, KT1 * F], F16, tag="w1")
            w1_v = w1_t[:].rearrange("p (kt f) -> p kt f", kt=KT1)
            w1_src = w1_d.rearrange("(kt p) f -> p kt f", p=P)
            w2_t = wpool.tile([P, KT2 * D], F16, tag="w2")
            b1_t = wpool.tile([P, FT], F32, tag="b1")
            cw_t = wpool.tile([P, Ct], F32, tag="cw")
            xt_t = xpool.tile([P, KT1 * C], F16, tag="xt")
            xt_v = xt_t[:].rearrange("p (kt c) -> p kt c", kt=KT1)
            xt_src = xt_d.rearrange("(kt p) c -> p kt c", p=P)

            # PE warm-up: a few junk matmuls on a zeroed tile while the input
            # DMAs stream, so the HAM clock-gate reaches 8/8 before real work
            # arrives and the first real matmuls don't run in the cold
            # 1.2 GHz window. Emitted before the DMA issues so the memset is
            # first in the GpSimd stream.
            warm = wpool.tile([P, 512], F16, tag="warm")
            nc.gpsimd.memset(warm[:], 0.0)
            wps = psy.tile([P, 512], F32, tag="psy")
            for _ in range(20):
                nc.tensor.matmul(wps[:], warm[:, 0:P], warm[:], start=True, stop=True)

            # Sync queue: what mm1 needs first (w1 quarters, then xt chunks,
            # interleaved so chunk-0 compute starts as early as possible).
            FQ = FT // 4
            def w1_dma(q):
                nc.sync.dma_start(
                    w1_v[:, :, q * FQ * P : (q + 1) * FQ * P],
                    w1_src[:, :, q * FQ * P : (q + 1) * FQ * P],
                )
            w1_dma(0)
            xt0_dma = nc.sync.dma_start(
                xt_v[:, :, 0 : chunks[0][1]], xt_src[:, :, 0 : chunks[0][1]]
            )
            for q in range(1, 4):
                w1_dma(q)
            for c0, S in chunks[1:]:
                nc.sync.dma_start(
                    xt_v[:, :, c0 : c0 + S], xt_src[:, :, c0 : c0 + S]
                )

            # GpSimd queue: everything whose deadline is later (b1 for the
            # first relu, w2 for mm2, cw for the y scale). w2 is big; gate it
            # on xt0's completion so it doesn't halve the HBM bandwidth
            # during the startup window the PE is waiting on.
            nc.gpsimd.dma_start(b1_t[:], b1_d[:])
            w2_dma = nc.gpsimd.dma_start(
                w2_t[:].rearrange("p (kt d) -> p kt d", kt=KT2),
                w2_d.rearrange("(kt p) d -> p kt d", p=P),
            )
            add_dep_helper(w2_dma.ins, xt0_dma.ins, sync=True,
                           reason="defer w2 until xt0 landed")
            nc.gpsimd.dma_start(cw_t[:], cw_d[:])

            # ---- software-pipelined chunk loop: mm1(ci) then mm2(ci-1) ----
            h_tiles = {}  # chunk idx -> list of FT hT tiles
            prev_grp = [None, None]  # previous group's first MM, current group's first MM

            def group_start():
                prev_grp[0], prev_grp[1] = prev_grp[1], None

            def chain(bi):
                # Pin PE group issue order to program order (first-MM to
                # first-MM): the scheduler otherwise reorders independent
                # matmul groups ahead of ready ones and stalls the PE on
                # not-yet-DMA'd data. Within-group order is already enforced
                # by PSUM accumulation, so leave those edges free for
                # LDWEIGHTS pull-ahead.
                if prev_grp[1] is None:
                    prev_grp[1] = bi
                    if prev_grp[0] is not None:
                        add_dep_helper(bi.ins, prev_grp[0].ins, sync=False,
                                       reason="PE group-order chain")

            def mm1(ci):
                c0, S = chunks[ci]
                tiles = []
                for fi in range(FT):
                    group_start()
                    ph = psh.tile([P, S], F32, tag="psh")
                    for kt in range(KT1):
                        chain(nc.tensor.matmul(
                            ph[:],
                            w1_t[:, kt * F + fi * P : kt * F + (fi + 1) * P],
                            xt_v[:, kt, c0 : c0 + S],
                            start=(kt == 0),
                            stop=(kt == KT1 - 1),
                        ))
                    ht = hpool.tile([P, S], F16, tag="h")
                    nc.scalar.activation(
                        ht[:],
                        ph[:],
                        mybir.ActivationFunctionType.Relu,
                        bias=b1_t[:, fi : fi + 1],
                    )
                    tiles.append(ht)
                h_tiles[ci] = tiles

            def mm2(ci):
                c0, S = chunks[ci]
                tiles = h_tiles.pop(ci)
                for mi in range(S // P):
                    group_start()
                    py = psy.tile([P, D], F32, tag="psy")
                    for kt in range(KT2):
                        chain(nc.tensor.matmul(
                            py[:],
                            tiles[kt][:, mi * P : (mi + 1) * P],
                            w2_t[:, kt * D : (kt + 1) * D],
                            start=(kt == 0),
                            stop=(kt == KT2 - 1),
                        ))
                    yt = ypool.tile([P, D], F32, tag="y")
                    ct = c0 // P + mi
                    nc.vector.tensor_scalar_mul(yt[:], py[:], cw_t[:, ct : ct + 1])
                    nc.gpsimd.dma_start(y_d[ct * P : (ct + 1) * P, :], yt[:])

            for ci in range(len(chunks) + 1):
                if ci < len(chunks):
                    mm1(ci)
                if ci >= 1:
                    mm2(ci - 1)

    nc.compile()
    return nc


def kernel(x, Wr, br, W1, b1, W2, b2):
    x = np.ascontiguousarray(np.asarray(x, np.float32))
    Wr = np.asarray(Wr, np.float32)
    br = np.asarray(br, np.float32)
    W1 = np.ascontiguousarray(np.asarray(W1, np.float32))
    b1 = np.ascontiguousarray(np.asarray(b1, np.float32))
    W2 = np.ascontiguousarray(np.asarray(W2, np.float32))
    b2 = np.asarray(b2, np.float32)

    xf = x.reshape(N, D)

    # ---- host router: softmax -> top-2 -> combine weights ----
    logits = xf @ Wr + br
    m = logits.max(axis=-1, keepdims=True)
    p = np.exp(logits - m, dtype=np.float32)
    p /= p.sum(axis=-1, keepdims=True)
    idx = np.argpartition(-p, TOPK - 1, axis=-1)[:, :TOPK]  # top-2 experts
    cw = np.zeros((N, E), np.float32)
    np.put_along_axis(cw, idx, np.take_along_axis(p, idx, axis=-1), axis=-1)

    tok = [np.nonzero(cw[:, e] > 0)[0] for e in range(E)]
    counts = [len(t) for t in tok]
    C = max(256, -(-max(counts) // 256) * 256)  # capacity, multiple of 256

    in_maps = []
    for e in range(E):
        te, ce = tok[e], counts[e]
        xt = np.zeros((D, C), np.float16)
        xt[:, :ce] = xf[te].T
        cwe = np.zeros((C,), np.float32)
        cwe[:ce] = cw[te, e]
        in_maps.append(
            {
                "xt": xt,
                "w1": np.ascontiguousarray(W1[e], np.float16),
                "w2": np.ascontiguousarray(W2[e], np.float16),
                "b1": np.ascontiguousarray(b1[e].reshape(FT, P).T),
                "cw": np.ascontiguousarray(cwe.reshape(C // P, P).T),
            }
        )

    nc = _build(C)
    trace = bool(os.environ.get("BASS_MOE_TRACE"))
    res = run_bass_kernel_spmd(
        nc,
        in_maps,
        core_ids=list(range(N_CORES)),
        trace=trace,
        trace_cores=list(range(N_CORES)) if trace else None,
    )
    if trace and res.exec_time_ns is not None:
        print(f"HW exec time: {res.exec_time_ns} ns")
        print(f"mean exec time: {res.mean_exec_time_ns} ns")
        if res.instructions_and_trace is not None:
            print(f"trace: {res.instructions_and_trace[1]}")

    # ---- host combine: scatter-add expert outputs + cw-weighted b2 ----
    out = cw @ b2  # (N, D) rank-E update: sum_e cw[:,e] * b2[e]
    for e in range(E):
        out[tok[e]] += res.results[e]["y"][: counts[e]]
    return out.reshape(B, T, D)
